# revision 1
# baseline (speedup 1.0000x reference)
"""Trainium2 Bass kernel for nn_DA_conv (dense_cnn).

Model (per batch element b, channels c):
  kern = leaky(d @ kW1.T) @ kW2.T            -> per-(b,c) 3x3 depthwise filter
  dw   = depthwise_conv3x3(x, kern), pad=1   (cross-correlation)
  act  = leaky(dw)
  out  = conv1x1(act, convW) + convB + x * sigmoid-attention(d)

Sharding: data-parallel over batch B=16 across 8 cores (2 images/core).
Per-core layout: 128 SBUF partitions = (2 images x 64 channels); the
spatial plane lives flat in the free dim with a 1-pixel zero border
(rows of 130), so every 3x3 tap is a pure free-dim offset read. x is
uploaded in bf16 (halves input DMA; the tap matmuls are bf16 anyway).

The image is processed in 11-row compute bands inside 22-row DMA tiles:
  - PE  : 9 depthwise taps as diag(kern)-matmuls (bf16, per-512-col PSUM
          chunk) + 1x1 conv as block-diag(convW.T) matmul + att*x folded
          in as a diag(att) matmul accumulating into the same PSUM chunk.
          10 dummy warm-up matmuls run during the serial MLP head so the
          HAM clock-gate is at 2.4 GHz when the real taps start.
  - ACT : leaky via Prelu(alpha=0.1) PSUM->SBUF (bf16 out), and the final
          PSUM evacuation with convB added as per-partition bias.
  - DVE : a few whole bands' depthwise via scalar_tensor_tensor tap
          chains (per-partition kern scalars) to offload the PE.
The tiny d-MLPs (kern, att) run on-device with fp32 matmuls, writing
kern/att directly in per-partition layout via per-image-half matmuls.
"""
import numpy as np
import ml_dtypes

import concourse.bacc as bacc
import concourse.bass as bass
import concourse.mybir as mybir
import concourse.tile as tile
from concourse.bass_utils import run_bass_kernel_spmd
from concourse.masks import make_identity

F32 = mybir.dt.float32
BF16 = mybir.dt.bfloat16
AF = mybir.ActivationFunctionType
ALU = mybir.AluOpType

B, C, H, W = 16, 64, 128, 128
NCORES = 8
BL = B // NCORES          # images per core (2)
P = BL * C                # partitions used (128)
WP = W + 2                # padded row length (130)
NEG = 0.1                 # leaky slope

BAND = 11                 # interior rows per band
DVE_BANDS = (2, 8, 11)    # bands whose depthwise runs on DVE instead of PE

_CACHE = {}


def _bands():
    out = []
    p0 = 0
    while p0 < H:
        nr = min(BAND, H - p0)
        out.append((p0, nr))
        p0 += nr
    return out


def _tiles(group=2):
    """Group sub-bands into DMA super-tiles of `group` bands each."""
    bands = _bands()
    out = []
    for i in range(0, len(bands), group):
        chunk = bands[i:i + group]
        row0 = chunk[0][0]
        nrows = sum(nr for _, nr in chunk)
        out.append((row0, nrows))
    return out


def _chunks(span):
    return [(cs, min(512, span - cs)) for cs in range(0, span, 512)]


def _build():
    nc = bacc.Bacc("TRN2", target_bir_lowering=False, debug=False)

    x_d = nc.dram_tensor("x", [BL, C, H, W], BF16, kind="ExternalInput")
    # packed [64, 650] = kW1T | kW2T | caW1T | dT  (all fp32, 64 rows)
    wpk_d = nc.dram_tensor("wpack", [C, 650], F32, kind="ExternalInput")
    caW2T_d = nc.dram_tensor("caW2T", [8, C], F32, kind="ExternalInput")
    cwbd_d = nc.dram_tensor("convWbd16", [P, P], BF16, kind="ExternalInput")
    cbf_d = nc.dram_tensor("convB2f", [P, 1], F32, kind="ExternalInput")
    out_d = nc.dram_tensor("out", [BL, C, H, W], F32, kind="ExternalOutput")

    with tile.TileContext(nc) as tc:
        with (
            tc.tile_pool(name="consts", bufs=1) as consts,
            tc.tile_pool(name="xb", bufs=4) as xbp,
            tc.tile_pool(name="actb", bufs=4) as actbp,
            tc.tile_pool(name="accb", bufs=3) as accbp,
            tc.tile_pool(name="outb", bufs=4) as outbp,
            tc.tile_pool(name="psA", bufs=4, space="PSUM") as psA,
            tc.tile_pool(name="psB", bufs=4, space="PSUM") as psB,
        ):
            # ---- load weights/inputs that persist ----
            wpk = consts.tile([C, 650], F32)
            caW2T = consts.tile([8, C], F32)
            cwbd = consts.tile([P, P], BF16)
            cbf = consts.tile([P, 1], F32)
            nc.sync.dma_start(out=wpk, in_=wpk_d.ap())
            nc.sync.dma_start(out=caW2T, in_=caW2T_d.ap())
            nc.sync.dma_start(out=cwbd, in_=cwbd_d.ap())
            nc.sync.dma_start(out=cbf, in_=cbf_d.ap())
            kW1T = wpk[:, 0:64]
            kW2T = wpk[:, 64:640]
            caW1T = wpk[:, 640:648]
            dT = wpk[:, 648:650]

            ident = consts.tile([P, P], F32)
            make_identity(nc, ident)
            # PE warm-up: dense dummy matmuls during the serial MLP head so
            # the HAM clock-gate reaches 2.4 GHz before the real taps start
            wps = psB.tile([P, 128], F32, tag="B")
            for _ in range(10):
                nc.tensor.matmul(wps, ident, ident, start=True, stop=True)

            # ---- kern MLP: kern = leaky(d @ kW1.T) @ kW2.T ----
            h1p = psB.tile([C, BL], F32, tag="B")
            nc.tensor.matmul(h1p, kW1T, dT, start=True, stop=True)
            h1 = consts.tile([C, BL], F32, tag="h1")
            nc.scalar.activation(h1, h1p, AF.Prelu, alpha=NEG)

            # kern in [(b,c), tap] layout directly: per tap t and image b,
            # out[64b:64b+64, t] = kW2[c*9+t, :] @ h1[:, b]
            kW2v = kW2T.rearrange("h (c t) -> h t c", t=9)
            kernp = psB.tile([P, 9], F32, tag="B")
            for t in range(9):
                for b in range(2):
                    nc.tensor.matmul(kernp[64 * b:64 * (b + 1), t:t + 1],
                                     kW2v[:, t, :], h1[:, b:b + 1],
                                     start=True, stop=True)
            kern_pp = consts.tile([P, 9], F32, tag="kern_pp")
            nc.scalar.copy(kern_pp, kernp)

            # diag tap matrices (bf16): diag16[:, t*128:(t+1)*128] = I * kern_t
            diag16 = consts.tile([P, 9 * P], BF16, tag="diag16")
            for t in range(9):
                nc.vector.tensor_scalar(diag16[:, P * t:P * (t + 1)], ident,
                                        kern_pp[:, t:t + 1], None, ALU.mult)

            # ---- attention MLP: att = sigmoid(leaky(d @ caW1.T) @ caW2.T) ----
            a1p = psB.tile([8, BL], F32, tag="B")
            nc.tensor.matmul(a1p, caW1T, dT, start=True, stop=True)
            a1 = consts.tile([8, BL], F32, tag="a1")
            nc.scalar.activation(a1, a1p, AF.Prelu, alpha=NEG)
            # att in [(b,o), 1] layout directly: one matmul per image half
            attp = psB.tile([P, 1], F32, tag="B")
            for b in range(2):
                nc.tensor.matmul(attp[64 * b:64 * (b + 1), 0:1],
                                 caW2T, a1[:, b:b + 1],
                                 start=True, stop=True)
            att_pp = consts.tile([P, 1], F32, tag="att_pp")
            nc.scalar.activation(att_pp, attp, AF.Sigmoid)
            attd16 = consts.tile([P, P], BF16, tag="attd16")
            nc.vector.tensor_scalar(attd16, ident, att_pp[:, 0:1], None,
                                    ALU.mult)

            # ---- main loop: DMA super-tiles of TROWS rows, compute
            # ---- sub-bands of BAND rows inside each tile ----
            bands = _bands()
            bi = 0
            for (row0, tnr) in _tiles():
                R = tnr + 2                   # padded rows in this DMA tile
                xb = xbp.tile([P, R * WP], BF16, tag="xb")
                xbv = xb.rearrange("p (r w) -> p r w", w=WP)
                # zero the left/right padding columns
                nc.gpsimd.memset(xbv[:, :, 0:1], 0.0)
                nc.gpsimd.memset(xbv[:, :, W + 1:W + 2], 0.0)
                # zero top/bottom padding rows (first/last tile only)
                r_lo = max(0, 1 - row0)
                r_hi = min(R, 129 - row0)
                if r_lo > 0:
                    nc.gpsimd.memset(xbv[:, 0:r_lo, 1:W + 1], 0.0)
                if r_hi < R:
                    nc.gpsimd.memset(xbv[:, r_hi:R, 1:W + 1], 0.0)
                nc.sync.dma_start(
                    out=xbv[:, r_lo:r_hi, 1:W + 1],
                    in_=x_d.ap().rearrange("b c h w -> (b c) h w")
                    [:, row0 + r_lo - 1:row0 + r_hi - 1, :])

                outb = outbp.tile([P, tnr * WP], F32, tag="outb")

                while bi < len(bands) and bands[bi][0] < row0 + tnr:
                    p0, nr = bands[bi]
                    span = (nr - 1) * WP + W
                    # offset of this sub-band's first interior output in xb
                    base = (p0 - row0 + 1) * WP + 1
                    obase = (p0 - row0) * WP   # ... and in outb

                    actb = actbp.tile([P, span], BF16, tag="actb")
                    if bi not in DVE_BANDS:
                        # PE depthwise: per 512-chunk, 9 diag matmuls into a
                        # single-bank PSUM accumulator, prelu per chunk
                        for (cs, wn) in _chunks(span):
                            pa = psA.tile([P, 512], F32, tag="A")
                            for t in range(9):
                                ky, kx = t // 3, t % 3
                                off = base + (ky - 1) * WP + (kx - 1) + cs
                                nc.tensor.matmul(
                                    pa[:, 0:wn],
                                    diag16[:, P * t:P * (t + 1)],
                                    xb[:, off:off + wn],
                                    start=(t == 0), stop=(t == 8))
                            nc.scalar.activation(actb[:, cs:cs + wn],
                                                 pa[:, 0:wn],
                                                 AF.Prelu, alpha=NEG)
                    else:
                        # DVE depthwise: scalar_tensor_tensor tap chain
                        acc = accbp.tile([P, span], F32, tag="acc")
                        for t in range(9):
                            ky, kx = t // 3, t % 3
                            off = base + (ky - 1) * WP + (kx - 1)
                            src = xb[:, off:off + span]
                            if t == 0:
                                nc.vector.tensor_scalar(
                                    acc, src, kern_pp[:, 0:1], None, ALU.mult)
                            else:
                                nc.vector.scalar_tensor_tensor(
                                    acc, src, kern_pp[:, t:t + 1], acc,
                                    op0=ALU.mult, op1=ALU.add)
                        nc.vector.scalar_tensor_tensor(
                            actb, acc, NEG, acc, op0=ALU.mult, op1=ALU.max)

                    # 1x1 conv + att*x residual into PSUM, evac on ACT
                    # with convB as per-partition bias
                    for (cs, wn) in _chunks(span):
                        pb = psB.tile([P, 512], F32, tag="B")
                        nc.tensor.matmul(pb[:, 0:wn], cwbd,
                                         actb[:, cs:cs + wn],
                                         start=True, stop=False)
                        nc.tensor.matmul(pb[:, 0:wn], attd16,
                                         xb[:, base + cs:base + cs + wn],
                                         start=False, stop=True)
                        nc.scalar.activation(
                            outb[:, obase + cs:obase + cs + wn], pb[:, 0:wn],
                            AF.Identity, bias=cbf[:, 0:1])
                    bi += 1

                nc.sync.dma_start(
                    out=out_d.ap().rearrange("b c h w -> (b c) h w")
                    [:, row0:row0 + tnr, :],
                    in_=outb.rearrange("p (r w) -> p r w", w=WP)[:, :, 0:W])

    nc.compile()
    return nc


def _prep_shared(kW1, kW2, convW, convB, caW1, caW2):
    cwbd = np.zeros((P, P), np.float32)
    cwbd[0:C, 0:C] = convW.T
    cwbd[C:P, C:P] = convW.T
    return {
        "caW2T": np.ascontiguousarray(caW2.T),
        "convWbd16": cwbd.astype(ml_dtypes.bfloat16),
        "convB2f": np.tile(convB, 2)[:, None].astype(np.float32),
    }


def kernel(x, d, kW1, kW2, convW, convB, caW1, caW2, _trace=False):
    x = np.asarray(x, np.float32).astype(ml_dtypes.bfloat16)
    d = np.asarray(d, np.float32)
    if "nc" not in _CACHE:
        _CACHE["nc"] = _build()
    nc = _CACHE["nc"]

    shared = _prep_shared(np.asarray(kW1, np.float32),
                          np.asarray(kW2, np.float32),
                          np.asarray(convW, np.float32),
                          np.asarray(convB, np.float32),
                          np.asarray(caW1, np.float32),
                          np.asarray(caW2, np.float32))
    kW1 = np.asarray(kW1, np.float32)
    kW2 = np.asarray(kW2, np.float32)
    caW1 = np.asarray(caW1, np.float32)
    in_maps = []
    for c in range(NCORES):
        sl = slice(c * BL, (c + 1) * BL)
        m = dict(shared)
        m["x"] = np.ascontiguousarray(x[sl])
        m["wpack"] = np.ascontiguousarray(
            np.concatenate([kW1.T, kW2.T, caW1.T, d[sl].T], axis=1))
        in_maps.append(m)

    last_err = None
    for _attempt in range(3):
        try:
            res = run_bass_kernel_spmd(nc, in_maps,
                                       core_ids=list(range(NCORES)),
                                       trace=_trace)
            break
        except Exception as e:  # transient NRT device errors recover on retry
            last_err = e
    else:
        raise last_err
    out = np.concatenate([r["out"] for r in res.results], axis=0)
    if _trace:
        return out, res
    return out



# revision 4
# speedup vs baseline: 1.8618x; 1.8618x over previous
"""Trainium2 Bass kernel for nn_DA_conv (dense_cnn).

Model (per batch element b, channel c):
  kern = leaky(d @ kW1.T) @ kW2.T            -> per-(b,c) 3x3 depthwise filter
  dw   = depthwise_conv3x3(x, kern), pad=1   (cross-correlation)
  act  = leaky(dw)
  out  = conv1x1(act, convW) + convB + x * sigmoid-attention(d)

Sharding: data-parallel over batch B=16 across 8 cores (2 images/core).
Per-core layout: 128 SBUF partitions = (2 images x 64 channels).

Strategy (fp8 DoubleRow everywhere on the PE):
- x is split on the host into x_hi = fp8(x) and x_lo = fp8(x - x_hi), both
  uploaded as zero-padded 130x130 planes (fully contiguous DMA, no device
  memsets). The tiny d-MLPs (kern, att) are computed on the host; their
  outputs become fp8 diagonal weight matrices uploaded directly.
- Depthwise: 9 taps = 5 DoubleRow matmuls per 512-px chunk (2 taps per
  matmul via the k-tile dim; moving AP k-tile stride = tap-offset delta).
  x_hi only; kern quantized to fp8 (validated: final rel err ~1.1e-2).
- conv1x1 + attention + residual in 3 DoubleRow matmuls per chunk:
    (convW_hi . a8, convW_res . a8)         stride-0 k-tile on a8
    (att8 . x_hi,   att8 . x_lo)            k-tile stride = plane
    (attres8 . x_hi, 0)                     fp8 residual of sigmoid att
  where a8 = fp8(leaky(dw)) and convW_hi/res is an fp8 hi/lo split.
- leaky+fp8-quant (prelu) and the final psum->bf16+bias evacuation are
  round-robined across ACT/DVE/Pool so no single engine bottlenecks.
- Output is written bf16 (halves the out-DMA; adds ~4e-4 rel rounding).
"""
import numpy as np
import ml_dtypes

import concourse.bacc as bacc
import concourse.bass as bass
import concourse.mybir as mybir
import concourse.tile as tile
from concourse.bass_utils import run_bass_kernel_spmd

F32 = mybir.dt.float32
BF16 = mybir.dt.bfloat16
F8 = mybir.dt.float8e4
NF8 = ml_dtypes.float8_e4m3
NBF = ml_dtypes.bfloat16
AF = mybir.ActivationFunctionType
ALU = mybir.AluOpType
PM = mybir.MatmulPerfMode.DoubleRow

B, C, H, W = 16, 64, 128, 128
NCORES = 8
BL = B // NCORES          # images per core (2)
P = BL * C                # partitions used (128)
WP = W + 2                # padded row length (130)
HPAD = H + 2              # padded rows (130)
PLANE = HPAD * WP         # 16900
NEG = 0.1                 # leaky slope

CB_ROWS = 16              # compute band rows
CH_ROWS = 4               # chunk rows (512 psum elements)
NBANDS = H // CB_ROWS     # 8
NCH = CB_ROWS // CH_ROWS  # 4 chunks per band
# padded-row DMA slices (disjoint, 2 per plane interleaved hi/lo)
IN_SLICES = [(0, 34), (34, 66), (66, 98), (98, 130)]

# depthwise tap pairs: (first tap, second tap); t = 3*(dy+1) + (dx+1)
TAP_PAIRS = [(0, 1), (2, 3), (4, 5), (6, 7), (8, 8)]

_CACHE = {}


def _tap_base(rr, t):
    dy, dx = t // 3 - 1, t % 3 - 1
    return (rr + 1 + dy) * WP + 1 + dx


def _subap(apx, off, dims):
    part = list(list(apx.ap)[0])
    return bass.AP(apx.tensor, apx.offset + off,
                   [part] + [list(d) for d in dims])


def _build():
    nc = bacc.Bacc("TRN2", target_bir_lowering=False, debug=False)

    xhl_d = nc.dram_tensor("xhl", [P, 2 * PLANE], F8, kind="ExternalInput")
    wdw_d = nc.dram_tensor("wdw", [P, 10 * P], F8, kind="ExternalInput")
    attw_d = nc.dram_tensor("attw", [P, 3 * P], F8, kind="ExternalInput")
    cwb8_d = nc.dram_tensor("cwb8", [P, 2 * P], F8, kind="ExternalInput")
    cbf_d = nc.dram_tensor("cbf", [P, 1], F32, kind="ExternalInput")
    out_d = nc.dram_tensor("out", [P, H * W], BF16, kind="ExternalOutput")

    with tile.TileContext(nc) as tc:
        with (
            tc.tile_pool(name="consts", bufs=1) as consts,
            tc.tile_pool(name="a8p", bufs=3) as a8p,
            tc.tile_pool(name="psA", bufs=3, space="PSUM") as psA,
            tc.tile_pool(name="psB", bufs=3, space="PSUM") as psB,
            tc.tile_pool(name="psW", bufs=2, space="PSUM") as psW,
        ):
            wdw = consts.tile([P, 10 * P], F8)
            attw = consts.tile([P, 3 * P], F8)
            cwb8 = consts.tile([P, 2 * P], F8)
            cbf = consts.tile([P, 1], F32)
            nc.sync.dma_start(out=cwb8, in_=cwb8_d.ap())
            nc.sync.dma_start(out=wdw, in_=wdw_d.ap())
            nc.sync.dma_start(out=attw, in_=attw_d.ap())
            nc.sync.dma_start(out=cbf, in_=cbf_d.ap())

            xhl = consts.tile([P, 2 * PLANE], F8)
            xap = xhl[:, :]
            for (a, b) in IN_SLICES:
                for q in range(2):
                    nc.sync.dma_start(
                        out=xhl[:, q * PLANE + a * WP: q * PLANE + b * WP],
                        in_=xhl_d.ap()[:, q * PLANE + a * WP:
                                       q * PLANE + b * WP])

            outb = consts.tile([P, H * W], BF16)

            # PE p-state warm-up: fp8 DoubleRow dummies on the (tiny) conv
            # weights so the clock is at 2.4 GHz when the real taps start.
            cw_k = cwb8[:, :].rearrange("p (k m) -> p k m", k=2)
            for i in range(40):
                wps = psW.tile([P, P], F32, tag="W")
                nc.tensor.matmul(wps, cw_k,
                                 _subap(cwb8[:, :], 0, [(P, 2), (1, P)]),
                                 start=True, stop=True, perf_mode=PM)

            # weight APs reused by every chunk
            w_dw = [wdw[:, 256 * i: 256 * (i + 1)]
                    .rearrange("p (k m) -> p k m", k=2) for i in range(5)]
            w_cv = cwb8[:, :].rearrange("p (k m) -> p k m", k=2)
            w_at = _subap(attw[:, :], 0, [(0, 2), (1, P)])
            w_ar = attw[:, P: 3 * P].rearrange("p (k m) -> p k m", k=2)

            # PSUM readers are limited: ACT does the prelu (psum->fp8),
            # DVE does the evacuation (psum->bf16 + bias). Pool/gpsimd
            # cannot read PSUM (walrus verifier rejects it).
            def leaky_to_fp8(dst, src, eng):
                nc.scalar.activation(dst, src, AF.Prelu, alpha=NEG)

            def evac_bias(dst, src, eng):
                nc.vector.tensor_scalar(dst, src, cbf[:, 0:1], None,
                                        ALU.add)

            for band in range(NBANDS):
                a8 = a8p.tile([P, CB_ROWS * W], F8, tag="a8")
                for j in range(NCH):
                    rr = band * CB_ROWS + j * CH_ROWS
                    # ---- depthwise: 5 DoubleRow tap-pair matmuls ----
                    psa = psA.tile([P, 512], F32, tag="A")
                    psa3 = psa.rearrange("p (r c) -> p r c", c=128)
                    for i, (ta, tb) in enumerate(TAP_PAIRS):
                        base = _tap_base(rr, ta)
                        s = _tap_base(rr, tb) - base
                        mv = _subap(xap, base,
                                    [(s, 2), (WP, CH_ROWS), (1, W)])
                        nc.tensor.matmul(psa3, w_dw[i], mv,
                                         start=(i == 0), stop=(i == 4),
                                         perf_mode=PM)
                    # ---- leaky -> fp8 ----
                    aslice = a8[:, j * 512: (j + 1) * 512]
                    leaky_to_fp8(aslice, psa, None)

                    # ---- conv1x1 + attention + residual ----
                    psb = psB.tile([P, 512], F32, tag="B")
                    psb3 = psb.rearrange("p (r c) -> p r c", c=128)
                    nc.tensor.matmul(psb, w_cv,
                                     _subap(a8[:, :], j * 512,
                                            [(0, 2), (1, 512)]),
                                     start=True, stop=False, perf_mode=PM)
                    abase = (rr + 1) * WP + 1
                    nc.tensor.matmul(psb3, w_at,
                                     _subap(xap, abase,
                                            [(PLANE, 2), (WP, CH_ROWS),
                                             (1, W)]),
                                     start=False, stop=False, perf_mode=PM)
                    nc.tensor.matmul(psb3, w_ar,
                                     _subap(xap, abase,
                                            [(0, 2), (WP, CH_ROWS), (1, W)]),
                                     start=False, stop=True, perf_mode=PM)
                    # ---- psum -> bf16 out (+convB) ----
                    evac_bias(outb[:, rr * W: rr * W + 512], psb, None)

                nc.sync.dma_start(
                    out=out_d.ap()[:, band * CB_ROWS * W:
                                   (band + 1) * CB_ROWS * W],
                    in_=outb[:, band * CB_ROWS * W: (band + 1) * CB_ROWS * W])

    nc.compile()
    return nc


def _leaky_np(v):
    return np.where(v >= 0, v, NEG * v)


def _prep_shared(d, kW1, kW2, convW, convB, caW1, caW2):
    # tiny per-sample MLPs on host: kern [B, C, 9], att [B, Cout]
    kern = (_leaky_np(d @ kW1.T) @ kW2.T).reshape(B, C, 9)
    att = 1.0 / (1.0 + np.exp(-(_leaky_np(d @ caW1.T) @ caW2.T)))

    cw8 = convW.astype(NF8).astype(np.float32)
    cwres8 = (convW - cw8).astype(NF8)
    cwb8 = np.zeros((P, 2 * P), NF8)
    for bi in range(BL):
        sl = slice(bi * C, (bi + 1) * C)
        cwb8[sl, bi * C:(bi + 1) * C] = cw8.T.astype(NF8)
        cwb8[sl, P + bi * C: P + (bi + 1) * C] = cwres8.T
    return kern, att, {
        "cwb8": cwb8,
        "cbf": np.tile(convB, BL)[:, None].astype(np.float32),
    }


def kernel(x, d, kW1, kW2, convW, convB, caW1, caW2, _trace=False):
    x = np.asarray(x, np.float32)
    d = np.asarray(d, np.float32)
    kW1 = np.asarray(kW1, np.float32)
    kW2 = np.asarray(kW2, np.float32)
    convW = np.asarray(convW, np.float32)
    convB = np.asarray(convB, np.float32)
    caW1 = np.asarray(caW1, np.float32)
    caW2 = np.asarray(caW2, np.float32)
    if "nc" not in _CACHE:
        _CACHE["nc"] = _build()
    nc = _CACHE["nc"]

    kern, att, shared = _prep_shared(d, kW1, kW2, convW, convB, caW1, caW2)

    xh = x.astype(NF8)
    xl = (x - xh.astype(np.float32)).astype(NF8)
    k8 = kern.astype(NF8)
    at8 = att.astype(NF8)
    atres8 = (att - at8.astype(np.float32)).astype(NF8)

    in_maps = []
    for c in range(NCORES):
        sl = slice(c * BL, (c + 1) * BL)
        m = dict(shared)
        xhl = np.zeros((P, 2, HPAD, WP), NF8)
        xhl[:, 0, 1:H + 1, 1:W + 1] = xh[sl].reshape(P, H, W)
        xhl[:, 1, 1:H + 1, 1:W + 1] = xl[sl].reshape(P, H, W)
        m["xhl"] = np.ascontiguousarray(xhl.reshape(P, 2 * PLANE))

        wdw = np.zeros((P, 10 * P), NF8)
        kc = k8[sl].reshape(P, 9)
        for t in range(9):
            wdw[np.arange(P), t * P + np.arange(P)] = kc[:, t]
        m["wdw"] = wdw

        attw = np.zeros((P, 3 * P), NF8)
        attw[np.arange(P), np.arange(P)] = at8[sl].reshape(P)
        attw[np.arange(P), P + np.arange(P)] = atres8[sl].reshape(P)
        m["attw"] = attw
        in_maps.append(m)

    last_err = None
    for _attempt in range(3):
        try:
            res = run_bass_kernel_spmd(nc, in_maps,
                                       core_ids=list(range(NCORES)),
                                       trace=_trace)
            break
        except Exception as e:  # transient NRT device errors recover on retry
            last_err = e
    else:
        raise last_err
    out = np.concatenate(
        [r["out"].astype(np.float32).reshape(BL, C, H, W)
         for r in res.results], axis=0)
    if _trace:
        return out, res
    return out


# revision 6
# speedup vs baseline: 1.8647x; 1.0016x over previous
"""Trainium2 Bass kernel for nn_DA_conv (dense_cnn).

Model (per batch element b, channel c):
  kern = leaky(d @ kW1.T) @ kW2.T            -> per-(b,c) 3x3 depthwise filter
  dw   = depthwise_conv3x3(x, kern), pad=1   (cross-correlation)
  act  = leaky(dw)
  out  = conv1x1(act, convW) + convB + x * sigmoid-attention(d)

Sharding: data-parallel over batch B=16 across 8 cores (2 images/core).
Per-core layout: 128 SBUF partitions = (2 images x 64 channels).

Strategy (fp8 DoubleRow everywhere on the PE):
- x is split on the host into x_hi = fp8(x) and x_lo = fp8(x - x_hi), both
  uploaded as zero-padded 130x130 planes (fully contiguous DMA, no device
  memsets). The tiny d-MLPs (kern, att) are computed on the host; their
  outputs become fp8 diagonal weight matrices uploaded directly.
- Depthwise: 9 taps = 5 DoubleRow matmuls per 512-px chunk (2 taps per
  matmul via the k-tile dim; moving AP k-tile stride = tap-offset delta).
  x_hi only; kern quantized to fp8 (validated: final rel err ~1.3e-2).
- conv1x1 + attention + residual in 3 DoubleRow matmuls per chunk:
    (convW_hi . a8, convW_res . a8)         stride-0 k-tile on a8
    (att8 . x_hi,   att8 . x_lo)            k-tile stride = plane
    (attres8 . x_hi, 0)                     fp8 residual of sigmoid att
  where a8 = fp8(leaky(dw)) and convW_hi/res is an fp8 hi/lo split.
- ACT does the prelu (psum->fp8), DVE the evacuation (psum->bf16 + bias);
  Pool/gpsimd cannot read PSUM here.
- DMA issue is spread across the SP (x planes) and ACT (weights+output)
  HWDGE queues; the first x slice is small so compute starts
  early; warm-up matmuls on an identity (no DMA dependency) ramp the PE
  p-state while the first DMAs land.
- Output is written bf16 (halves the out-DMA; adds ~4e-4 rel rounding).
"""
import numpy as np
import ml_dtypes

import concourse.bacc as bacc
import concourse.bass as bass
import concourse.mybir as mybir
import concourse.tile as tile
from concourse.bass_utils import run_bass_kernel_spmd
from concourse.masks import make_identity

F32 = mybir.dt.float32
BF16 = mybir.dt.bfloat16
F8 = mybir.dt.float8e4
NF8 = ml_dtypes.float8_e4m3
AF = mybir.ActivationFunctionType
ALU = mybir.AluOpType
PM = mybir.MatmulPerfMode.DoubleRow

B, C, H, W = 16, 64, 128, 128
NCORES = 8
BL = B // NCORES          # images per core (2)
P = BL * C                # partitions used (128)
WP = W + 2                # padded row length (130)
HPAD = H + 2              # padded rows (130)
PLANE = HPAD * WP         # 16900
NEG = 0.1                 # leaky slope

CB_ROWS = 16              # compute band rows
CH_ROWS = 4               # chunk rows (512 psum elements)
NBANDS = H // CB_ROWS     # 8
NCH = CB_ROWS // CH_ROWS  # 4 chunks per band
# padded-row DMA slices (disjoint; first one small so compute starts early)
IN_SLICES = [(0, 18), (18, 46), (46, 78), (78, 110), (110, 130)]
NWARM = 48                # PE p-state warm-up matmuls (64 cols each)

# packed fp8 weights blob layout: wdw | attw | cwb8
WDW_O, ATTW_O, CWB_O = 0, 10 * P, 13 * P
WPK_COLS = 15 * P

# depthwise tap pairs: (first tap, second tap); t = 3*(dy+1) + (dx+1)
TAP_PAIRS = [(0, 1), (2, 3), (4, 5), (6, 7), (8, 8)]

_CACHE = {}


def _tap_base(rr, t):
    dy, dx = t // 3 - 1, t % 3 - 1
    return (rr + 1 + dy) * WP + 1 + dx


def _subap(apx, off, dims):
    part = list(list(apx.ap)[0])
    return bass.AP(apx.tensor, apx.offset + off,
                   [part] + [list(d) for d in dims])


def _build():
    nc = bacc.Bacc("TRN2", target_bir_lowering=False, debug=False)

    xhl_d = nc.dram_tensor("xhl", [P, 2 * PLANE], F8, kind="ExternalInput")
    wpk_d = nc.dram_tensor("wpk8", [P, WPK_COLS], F8, kind="ExternalInput")
    cbf_d = nc.dram_tensor("cbf", [P, 1], F32, kind="ExternalInput")
    out_d = nc.dram_tensor("out", [P, H * W], BF16, kind="ExternalOutput")

    with tile.TileContext(nc) as tc:
        with (
            tc.tile_pool(name="consts", bufs=1) as consts,
            tc.tile_pool(name="a8p", bufs=4) as a8p,
            tc.tile_pool(name="psA", bufs=3, space="PSUM") as psA,
            tc.tile_pool(name="psB", bufs=3, space="PSUM") as psB,
            tc.tile_pool(name="psW", bufs=1, space="PSUM") as psW,
        ):
            xhl = consts.tile([P, 2 * PLANE], F8)
            xap = xhl[:, :]
            wpk = consts.tile([P, WPK_COLS], F8)
            cbf = consts.tile([P, 1], F32)
            ident = consts.tile([P, P], BF16)
            outb = consts.tile([P, H * W], BF16)

            # first x slice on SP, weights on DVE queue: both in flight at t~0
            (a0, b0) = IN_SLICES[0]
            nc.sync.dma_start(out=xhl[:, a0 * WP: b0 * WP],
                              in_=xhl_d.ap()[:, a0 * WP: b0 * WP])
            nc.scalar.dma_start(out=wpk, in_=wpk_d.ap())
            nc.scalar.dma_start(out=cbf, in_=cbf_d.ap())
            nc.sync.dma_start(
                out=xhl[:, PLANE + a0 * WP: PLANE + b0 * WP],
                in_=xhl_d.ap()[:, PLANE + a0 * WP: PLANE + b0 * WP])
            for (a, b) in IN_SLICES[1:]:
                for q in range(2):
                    nc.sync.dma_start(
                        out=xhl[:, q * PLANE + a * WP: q * PLANE + b * WP],
                        in_=xhl_d.ap()[:, q * PLANE + a * WP:
                                       q * PLANE + b * WP])

            # PE p-state warm-up: bf16 matmuls on the identity, all into one
            # PSUM tile (same-engine WAW, no semaphore gaps), no DMA deps.
            make_identity(nc, ident)
            wps = psW.tile([P, 64], F32)
            for _ in range(NWARM):
                nc.tensor.matmul(wps, ident, ident[:, 0:64],
                                 start=True, stop=True)

            # weight APs reused by every chunk
            w_dw = [wpk[:, WDW_O + 256 * i: WDW_O + 256 * (i + 1)]
                    .rearrange("p (k m) -> p k m", k=2) for i in range(5)]
            w_cv = wpk[:, CWB_O: CWB_O + 2 * P].rearrange(
                "p (k m) -> p k m", k=2)
            w_at = _subap(wpk[:, :], ATTW_O, [(0, 2), (1, P)])
            w_ar = wpk[:, ATTW_O + P: ATTW_O + 3 * P].rearrange(
                "p (k m) -> p k m", k=2)

            for band in range(NBANDS):
                a8 = a8p.tile([P, CB_ROWS * W], F8, tag="a8")
                for j in range(NCH):
                    rr = band * CB_ROWS + j * CH_ROWS
                    # ---- depthwise: 5 DoubleRow tap-pair matmuls ----
                    psa = psA.tile([P, 512], F32, tag="A")
                    psa3 = psa.rearrange("p (r c) -> p r c", c=128)
                    for i, (ta, tb) in enumerate(TAP_PAIRS):
                        base = _tap_base(rr, ta)
                        s = _tap_base(rr, tb) - base
                        mv = _subap(xap, base,
                                    [(s, 2), (WP, CH_ROWS), (1, W)])
                        nc.tensor.matmul(psa3, w_dw[i], mv,
                                         start=(i == 0), stop=(i == 4),
                                         perf_mode=PM)
                    # ---- leaky -> fp8 (ACT) ----
                    nc.scalar.activation(a8[:, j * 512: (j + 1) * 512], psa,
                                         AF.Prelu, alpha=NEG)

                    # ---- conv1x1 + attention + residual ----
                    psb = psB.tile([P, 512], F32, tag="B")
                    psb3 = psb.rearrange("p (r c) -> p r c", c=128)
                    nc.tensor.matmul(psb, w_cv,
                                     _subap(a8[:, :], j * 512,
                                            [(0, 2), (1, 512)]),
                                     start=True, stop=False, perf_mode=PM)
                    abase = (rr + 1) * WP + 1
                    nc.tensor.matmul(psb3, w_at,
                                     _subap(xap, abase,
                                            [(PLANE, 2), (WP, CH_ROWS),
                                             (1, W)]),
                                     start=False, stop=False, perf_mode=PM)
                    nc.tensor.matmul(psb3, w_ar,
                                     _subap(xap, abase,
                                            [(0, 2), (WP, CH_ROWS), (1, W)]),
                                     start=False, stop=True, perf_mode=PM)
                    # ---- psum -> bf16 out (+convB) on DVE ----
                    nc.vector.tensor_scalar(outb[:, rr * W: rr * W + 512],
                                            psb, cbf[:, 0:1], None, ALU.add)
                    # half-band output DMA on the ACT queue
                    if j % 2 == 1:
                        o0 = (rr - CH_ROWS) * W
                        nc.scalar.dma_start(
                            out=out_d.ap()[:, o0: o0 + 1024],
                            in_=outb[:, o0: o0 + 1024])

    nc.compile()
    return nc


def _leaky_np(v):
    return np.where(v >= 0, v, NEG * v)


def kernel(x, d, kW1, kW2, convW, convB, caW1, caW2, _trace=False):
    x = np.asarray(x, np.float32)
    d = np.asarray(d, np.float32)
    kW1 = np.asarray(kW1, np.float32)
    kW2 = np.asarray(kW2, np.float32)
    convW = np.asarray(convW, np.float32)
    convB = np.asarray(convB, np.float32)
    caW1 = np.asarray(caW1, np.float32)
    caW2 = np.asarray(caW2, np.float32)
    if "nc" not in _CACHE:
        _CACHE["nc"] = _build()
    nc = _CACHE["nc"]

    # tiny per-sample MLPs on host: kern [B, C, 9], att [B, Cout]
    kern = (_leaky_np(d @ kW1.T) @ kW2.T).reshape(B, C, 9)
    att = 1.0 / (1.0 + np.exp(-(_leaky_np(d @ caW1.T) @ caW2.T)))

    cw8 = convW.astype(NF8).astype(np.float32)
    cwres8 = (convW - cw8).astype(NF8)
    cwb8 = np.zeros((P, 2 * P), NF8)
    for bi in range(BL):
        sl = slice(bi * C, (bi + 1) * C)
        cwb8[sl, bi * C:(bi + 1) * C] = cw8.T.astype(NF8)
        cwb8[sl, P + bi * C: P + (bi + 1) * C] = cwres8.T
    cbf = np.tile(convB, BL)[:, None].astype(np.float32)

    xh = x.astype(NF8)
    xl = (x - xh.astype(np.float32)).astype(NF8)
    k8 = kern.astype(NF8)
    at8 = att.astype(NF8)
    atres8 = (att - at8.astype(np.float32)).astype(NF8)

    in_maps = []
    rng = np.arange(P)
    for c in range(NCORES):
        sl = slice(c * BL, (c + 1) * BL)
        xhl = np.zeros((P, 2, HPAD, WP), NF8)
        xhl[:, 0, 1:H + 1, 1:W + 1] = xh[sl].reshape(P, H, W)
        xhl[:, 1, 1:H + 1, 1:W + 1] = xl[sl].reshape(P, H, W)

        wpk = np.zeros((P, WPK_COLS), NF8)
        kc = k8[sl].reshape(P, 9)
        for t in range(9):
            wpk[rng, WDW_O + t * P + rng] = kc[:, t]
        wpk[rng, ATTW_O + rng] = at8[sl].reshape(P)
        wpk[rng, ATTW_O + P + rng] = atres8[sl].reshape(P)
        wpk[:, CWB_O: CWB_O + 2 * P] = cwb8

        in_maps.append({
            "xhl": np.ascontiguousarray(xhl.reshape(P, 2 * PLANE)),
            "wpk8": wpk,
            "cbf": cbf,
        })

    last_err = None
    for _attempt in range(3):
        try:
            res = run_bass_kernel_spmd(nc, in_maps,
                                       core_ids=list(range(NCORES)),
                                       trace=_trace)
            break
        except Exception as e:  # transient NRT device errors recover on retry
            last_err = e
    else:
        raise last_err
    out = np.concatenate(
        [r["out"].astype(np.float32).reshape(BL, C, H, W)
         for r in res.results], axis=0)
    if _trace:
        return out, res
    return out


# revision 7
# speedup vs baseline: 1.9776x; 1.0606x over previous
"""Trainium2 Bass kernel for nn_DA_conv (dense_cnn).

Model (per batch element b, channel c):
  kern = leaky(d @ kW1.T) @ kW2.T            -> per-(b,c) 3x3 depthwise filter
  dw   = depthwise_conv3x3(x, kern), pad=1   (cross-correlation)
  act  = leaky(dw)
  out  = conv1x1(act, convW) + convB + x * sigmoid-attention(d)

Sharding: data-parallel over batch B=16 across 8 cores (2 images/core).
Per-core layout: 128 SBUF partitions = (2 images x 64 channels).

Strategy (fp8 DoubleRow everywhere on the PE):
- x is split on the host into x_hi = fp8(x) and x_lo = fp8(x - x_hi), both
  uploaded as zero-padded 130x130 planes (fully contiguous DMA, no device
  memsets). The tiny d-MLPs (kern, att) are computed on the host; their
  outputs become fp8 diagonal weight matrices uploaded directly.
- Depthwise: 9 taps = 5 DoubleRow matmuls per 512-px chunk (2 taps per
  matmul via the k-tile dim; moving AP k-tile stride = tap-offset delta).
  x_hi only; kern quantized to fp8 (validated: final rel err ~1.3e-2).
- conv1x1 + attention + residual in 3 DoubleRow matmuls per chunk:
    (convW_hi . a8, convW_res . a8)         stride-0 k-tile on a8
    (att8 . x_hi,   att8 . x_lo)            k-tile stride = plane
    (attres8 . x_hi, 0)                     fp8 residual of sigmoid att
  where a8 = fp8(leaky(dw)) and convW_hi/res is an fp8 hi/lo split.
- ACT does the prelu (psum->fp8), DVE the evacuation (psum->bf16 + bias);
  Pool/gpsimd cannot read PSUM here.
- DMA issue: x planes on SP, weights on ACT, outputs on the gpsimd/Pool
  SWDGE queue (a waiting DMA holds its queue's sequencer, so outputs must
  not share a queue with compute dispatch); the first x slice is small so compute starts
  early; warm-up matmuls on an identity (no DMA dependency) ramp the PE
  p-state while the first DMAs land.
- Output is written bf16 (halves the out-DMA; adds ~4e-4 rel rounding).
"""
import numpy as np
import ml_dtypes

import concourse.bacc as bacc
import concourse.bass as bass
import concourse.mybir as mybir
import concourse.tile as tile
from concourse.bass_utils import run_bass_kernel_spmd
from concourse.masks import make_identity

F32 = mybir.dt.float32
BF16 = mybir.dt.bfloat16
F8 = mybir.dt.float8e4
NF8 = ml_dtypes.float8_e4m3
AF = mybir.ActivationFunctionType
ALU = mybir.AluOpType
PM = mybir.MatmulPerfMode.DoubleRow

B, C, H, W = 16, 64, 128, 128
NCORES = 8
BL = B // NCORES          # images per core (2)
P = BL * C                # partitions used (128)
WP = W + 2                # padded row length (130)
HPAD = H + 2              # padded rows (130)
PLANE = HPAD * WP         # 16900
NEG = 0.1                 # leaky slope

CB_ROWS = 16              # compute band rows
CH_ROWS = 4               # chunk rows (512 psum elements)
NBANDS = H // CB_ROWS     # 8
NCH = CB_ROWS // CH_ROWS  # 4 chunks per band
# padded-row DMA slices (disjoint; first one small so compute starts early)
IN_SLICES = [(0, 12), (12, 34), (34, 56), (56, 78), (78, 100),
             (100, 122), (122, 130)]
NWARM = 36                # PE p-state warm-up matmuls (64 cols each)

# packed fp8 weights blob layout: wdw | attw | cwb8
WDW_O, ATTW_O, CWB_O = 0, 10 * P, 13 * P
WPK_COLS = 15 * P

# depthwise tap pairs: (first tap, second tap); t = 3*(dy+1) + (dx+1)
TAP_PAIRS = [(0, 1), (2, 3), (4, 5), (6, 7), (8, 8)]

_CACHE = {}


def _tap_base(rr, t):
    dy, dx = t // 3 - 1, t % 3 - 1
    return (rr + 1 + dy) * WP + 1 + dx


def _subap(apx, off, dims):
    part = list(list(apx.ap)[0])
    return bass.AP(apx.tensor, apx.offset + off,
                   [part] + [list(d) for d in dims])


def _build():
    nc = bacc.Bacc("TRN2", target_bir_lowering=False, debug=False)

    xhl_d = nc.dram_tensor("xhl", [P, 2 * PLANE], F8, kind="ExternalInput")
    wpk_d = nc.dram_tensor("wpk8", [P, WPK_COLS], F8, kind="ExternalInput")
    cbf_d = nc.dram_tensor("cbf", [P, 1], F32, kind="ExternalInput")
    out_d = nc.dram_tensor("out", [P, H * W], BF16, kind="ExternalOutput")

    with tile.TileContext(nc) as tc:
        with (
            tc.tile_pool(name="consts", bufs=1) as consts,
            tc.tile_pool(name="a8p", bufs=4) as a8p,
            tc.tile_pool(name="psA", bufs=3, space="PSUM") as psA,
            tc.tile_pool(name="psB", bufs=4, space="PSUM") as psB,
            tc.tile_pool(name="psW", bufs=1, space="PSUM") as psW,
        ):
            xhl = consts.tile([P, 2 * PLANE], F8)
            xap = xhl[:, :]
            wpk = consts.tile([P, WPK_COLS], F8)
            cbf = consts.tile([P, 1], F32)
            ident = consts.tile([P, P], BF16)
            outb = consts.tile([P, H * W], BF16)

            # first x slice on SP, weights on DVE queue: both in flight at t~0
            (a0, b0) = IN_SLICES[0]
            nc.sync.dma_start(out=xhl[:, a0 * WP: b0 * WP],
                              in_=xhl_d.ap()[:, a0 * WP: b0 * WP])
            nc.scalar.dma_start(out=wpk, in_=wpk_d.ap())
            nc.scalar.dma_start(out=cbf, in_=cbf_d.ap())
            nc.sync.dma_start(
                out=xhl[:, PLANE + a0 * WP: PLANE + b0 * WP],
                in_=xhl_d.ap()[:, PLANE + a0 * WP: PLANE + b0 * WP])
            for (a, b) in IN_SLICES[1:]:
                for q in range(2):
                    nc.sync.dma_start(
                        out=xhl[:, q * PLANE + a * WP: q * PLANE + b * WP],
                        in_=xhl_d.ap()[:, q * PLANE + a * WP:
                                       q * PLANE + b * WP])

            # PE p-state warm-up: bf16 matmuls on the identity, all into one
            # PSUM tile (same-engine WAW, no semaphore gaps), no DMA deps.
            make_identity(nc, ident)
            wps = psW.tile([P, 64], F32)
            for _ in range(NWARM):
                nc.tensor.matmul(wps, ident, ident[:, 0:64],
                                 start=True, stop=True)

            # weight APs reused by every chunk
            w_dw = [wpk[:, WDW_O + 256 * i: WDW_O + 256 * (i + 1)]
                    .rearrange("p (k m) -> p k m", k=2) for i in range(5)]
            w_cv = wpk[:, CWB_O: CWB_O + 2 * P].rearrange(
                "p (k m) -> p k m", k=2)
            w_at = _subap(wpk[:, :], ATTW_O, [(0, 2), (1, P)])
            w_ar = wpk[:, ATTW_O + P: ATTW_O + 3 * P].rearrange(
                "p (k m) -> p k m", k=2)

            for band in range(NBANDS):
                a8 = a8p.tile([P, CB_ROWS * W], F8, tag="a8")
                for j in range(NCH):
                    rr = band * CB_ROWS + j * CH_ROWS
                    # ---- depthwise: 5 DoubleRow tap-pair matmuls ----
                    psa = psA.tile([P, 512], F32, tag="A")
                    psa3 = psa.rearrange("p (r c) -> p r c", c=128)
                    for i, (ta, tb) in enumerate(TAP_PAIRS):
                        base = _tap_base(rr, ta)
                        s = _tap_base(rr, tb) - base
                        mv = _subap(xap, base,
                                    [(s, 2), (WP, CH_ROWS), (1, W)])
                        nc.tensor.matmul(psa3, w_dw[i], mv,
                                         start=(i == 0), stop=(i == 4),
                                         perf_mode=PM)
                    # ---- leaky -> fp8 (ACT) ----
                    nc.scalar.activation(a8[:, j * 512: (j + 1) * 512], psa,
                                         AF.Prelu, alpha=NEG)

                    # ---- conv1x1 + attention + residual ----
                    psb = psB.tile([P, 512], F32, tag="B")
                    psb3 = psb.rearrange("p (r c) -> p r c", c=128)
                    nc.tensor.matmul(psb, w_cv,
                                     _subap(a8[:, :], j * 512,
                                            [(0, 2), (1, 512)]),
                                     start=True, stop=False, perf_mode=PM)
                    abase = (rr + 1) * WP + 1
                    nc.tensor.matmul(psb3, w_at,
                                     _subap(xap, abase,
                                            [(PLANE, 2), (WP, CH_ROWS),
                                             (1, W)]),
                                     start=False, stop=False, perf_mode=PM)
                    nc.tensor.matmul(psb3, w_ar,
                                     _subap(xap, abase,
                                            [(0, 2), (WP, CH_ROWS), (1, W)]),
                                     start=False, stop=True, perf_mode=PM)
                    # ---- psum -> bf16 out (+convB) on DVE ----
                    nc.vector.tensor_scalar(outb[:, rr * W: rr * W + 512],
                                            psb, cbf[:, 0:1], None, ALU.add)
                    # half-band output DMA on the (otherwise idle) Pool
                    # queue: a waiting DMA blocks its queue's sequencer, so
                    # it must not share a queue with compute dispatch.
                    if j % 2 == 1:
                        o0 = (rr - CH_ROWS) * W
                        nc.gpsimd.dma_start(
                            out=out_d.ap()[:, o0: o0 + 1024],
                            in_=outb[:, o0: o0 + 1024])

    nc.compile()
    return nc


def _leaky_np(v):
    return np.where(v >= 0, v, NEG * v)


def kernel(x, d, kW1, kW2, convW, convB, caW1, caW2, _trace=False):
    x = np.asarray(x, np.float32)
    d = np.asarray(d, np.float32)
    kW1 = np.asarray(kW1, np.float32)
    kW2 = np.asarray(kW2, np.float32)
    convW = np.asarray(convW, np.float32)
    convB = np.asarray(convB, np.float32)
    caW1 = np.asarray(caW1, np.float32)
    caW2 = np.asarray(caW2, np.float32)
    if "nc" not in _CACHE:
        _CACHE["nc"] = _build()
    nc = _CACHE["nc"]

    # tiny per-sample MLPs on host: kern [B, C, 9], att [B, Cout]
    kern = (_leaky_np(d @ kW1.T) @ kW2.T).reshape(B, C, 9)
    att = 1.0 / (1.0 + np.exp(-(_leaky_np(d @ caW1.T) @ caW2.T)))

    cw8 = convW.astype(NF8).astype(np.float32)
    cwres8 = (convW - cw8).astype(NF8)
    cwb8 = np.zeros((P, 2 * P), NF8)
    for bi in range(BL):
        sl = slice(bi * C, (bi + 1) * C)
        cwb8[sl, bi * C:(bi + 1) * C] = cw8.T.astype(NF8)
        cwb8[sl, P + bi * C: P + (bi + 1) * C] = cwres8.T
    cbf = np.tile(convB, BL)[:, None].astype(np.float32)

    xh = x.astype(NF8)
    xl = (x - xh.astype(np.float32)).astype(NF8)
    k8 = kern.astype(NF8)
    at8 = att.astype(NF8)
    atres8 = (att - at8.astype(np.float32)).astype(NF8)

    in_maps = []
    rng = np.arange(P)
    for c in range(NCORES):
        sl = slice(c * BL, (c + 1) * BL)
        xhl = np.zeros((P, 2, HPAD, WP), NF8)
        xhl[:, 0, 1:H + 1, 1:W + 1] = xh[sl].reshape(P, H, W)
        xhl[:, 1, 1:H + 1, 1:W + 1] = xl[sl].reshape(P, H, W)

        wpk = np.zeros((P, WPK_COLS), NF8)
        kc = k8[sl].reshape(P, 9)
        for t in range(9):
            wpk[rng, WDW_O + t * P + rng] = kc[:, t]
        wpk[rng, ATTW_O + rng] = at8[sl].reshape(P)
        wpk[rng, ATTW_O + P + rng] = atres8[sl].reshape(P)
        wpk[:, CWB_O: CWB_O + 2 * P] = cwb8

        in_maps.append({
            "xhl": np.ascontiguousarray(xhl.reshape(P, 2 * PLANE)),
            "wpk8": wpk,
            "cbf": cbf,
        })

    last_err = None
    for _attempt in range(3):
        try:
            res = run_bass_kernel_spmd(nc, in_maps,
                                       core_ids=list(range(NCORES)),
                                       trace=_trace)
            break
        except Exception as e:  # transient NRT device errors recover on retry
            last_err = e
    else:
        raise last_err
    out = np.concatenate(
        [r["out"].astype(np.float32).reshape(BL, C, H, W)
         for r in res.results], axis=0)
    if _trace:
        return out, res
    return out


# revision 8
# speedup vs baseline: 2.0097x; 1.0162x over previous
"""Trainium2 Bass kernel for nn_DA_conv (dense_cnn).

Model (per batch element b, channel c):
  kern = leaky(d @ kW1.T) @ kW2.T            -> per-(b,c) 3x3 depthwise filter
  dw   = depthwise_conv3x3(x, kern), pad=1   (cross-correlation)
  act  = leaky(dw)
  out  = conv1x1(act, convW) + convB + x * sigmoid-attention(d)

Sharding: data-parallel over batch B=16 across 8 cores (2 images/core).
Per-core layout: 128 SBUF partitions = (2 images x 64 channels).

Strategy (fp8 DoubleRow everywhere on the PE):
- x is split on the host into x_hi = fp8(x) and x_lo = fp8(x - x_hi), both
  uploaded as zero-padded 130x130 planes (fully contiguous DMA, no device
  memsets). The tiny d-MLPs (kern, att) are computed on the host; their
  outputs become fp8 diagonal weight matrices uploaded directly.
- Depthwise: 9 taps = 5 DoubleRow matmuls per 512-px chunk (2 taps per
  matmul via the k-tile dim; moving AP k-tile stride = tap-offset delta).
  x_hi only; kern quantized to fp8 (validated: final rel err ~1.3e-2).
- conv1x1 + attention + residual in 3 DoubleRow matmuls per chunk:
    (convW_hi . a8, convW_res . a8)         stride-0 k-tile on a8
    (att8 . x_hi,   att8 . x_lo)            k-tile stride = plane
    (attres8 . x_hi, 0)                     fp8 residual of sigmoid att
  where a8 = fp8(leaky(dw)) and convW_hi/res is an fp8 hi/lo split.
- ACT does the prelu (psum->fp8), DVE the evacuation (psum->bf16 + bias);
  Pool/gpsimd cannot read PSUM here.
- DMA issue: x planes on SP, weights on ACT, outputs on the gpsimd/Pool
  SWDGE queue (a waiting DMA holds its queue's sequencer, so outputs must
  not share a queue with compute dispatch); the first x slice is small so compute starts
  early; warm-up matmuls on an identity (no DMA dependency) ramp the PE
  p-state while the first DMAs land.
- Output is written bf16 (halves the out-DMA; adds ~4e-4 rel rounding).
"""
import numpy as np
import ml_dtypes

import concourse.bacc as bacc
import concourse.bass as bass
import concourse.mybir as mybir
import concourse.tile as tile
from concourse.bass_utils import run_bass_kernel_spmd
from concourse.masks import make_identity

F32 = mybir.dt.float32
BF16 = mybir.dt.bfloat16
F8 = mybir.dt.float8e4
NF8 = ml_dtypes.float8_e4m3
AF = mybir.ActivationFunctionType
ALU = mybir.AluOpType
PM = mybir.MatmulPerfMode.DoubleRow

B, C, H, W = 16, 64, 128, 128
NCORES = 8
BL = B // NCORES          # images per core (2)
P = BL * C                # partitions used (128)
WP = W + 2                # padded row length (130)
HPAD = H + 2              # padded rows (130)
PLANE = HPAD * WP         # 16900
NEG = 0.1                 # leaky slope

CB_ROWS = 16              # compute band rows
CH_ROWS = 4               # chunk rows (512 psum elements)
NBANDS = H // CB_ROWS     # 8
NCH = CB_ROWS // CH_ROWS  # 4 chunks per band
# padded-row DMA slices (disjoint; first one small so compute starts early)
IN_SLICES = [(0, 12), (12, 34), (34, 56), (56, 78), (78, 100),
             (100, 122), (122, 130)]
NWARM = 50                # PE p-state warm-up matmuls (64 cols each)

# packed fp8 weights blob layout: wdw | attw | cwb8
WDW_O, ATTW_O, CWB_O = 0, 10 * P, 13 * P
WPK_COLS = 15 * P

# depthwise tap pairs: (first tap, second tap); t = 3*(dy+1) + (dx+1)
TAP_PAIRS = [(0, 1), (2, 3), (4, 5), (6, 7), (8, 8)]

_CACHE = {}


def _tap_base(rr, t):
    dy, dx = t // 3 - 1, t % 3 - 1
    return (rr + 1 + dy) * WP + 1 + dx


def _subap(apx, off, dims):
    part = list(list(apx.ap)[0])
    return bass.AP(apx.tensor, apx.offset + off,
                   [part] + [list(d) for d in dims])


def _build():
    nc = bacc.Bacc("TRN2", target_bir_lowering=False, debug=False)

    xhl_d = nc.dram_tensor("xhl", [P, 2 * PLANE], F8, kind="ExternalInput")
    wpk_d = nc.dram_tensor("wpk8", [P, WPK_COLS], F8, kind="ExternalInput")
    cbf_d = nc.dram_tensor("cbf", [P, 1], F32, kind="ExternalInput")
    out_d = nc.dram_tensor("out", [P, H * W], BF16, kind="ExternalOutput")

    with tile.TileContext(nc) as tc:
        with (
            tc.tile_pool(name="consts", bufs=1) as consts,
            tc.tile_pool(name="a8p", bufs=4) as a8p,
            tc.tile_pool(name="psA", bufs=3, space="PSUM") as psA,
            tc.tile_pool(name="psB", bufs=4, space="PSUM") as psB,
            tc.tile_pool(name="psW", bufs=1, space="PSUM") as psW,
        ):
            xhl = consts.tile([P, 2 * PLANE], F8)
            xap = xhl[:, :]
            wpk = consts.tile([P, WPK_COLS], F8)
            cbf = consts.tile([P, 1], F32)
            ident = consts.tile([P, P], BF16)
            outb = consts.tile([P, H * W], BF16)

            # weights first (they gate the first matmul), then x slices.
            # hi slices run one ahead of lo: the depthwise only needs hi,
            # the conv/att pass needs lo slightly later.
            nc.sync.dma_start(out=wpk, in_=wpk_d.ap())
            nc.sync.dma_start(out=cbf, in_=cbf_d.ap())

            def xslice(q, k):
                (a, b) = IN_SLICES[k]
                nc.sync.dma_start(
                    out=xhl[:, q * PLANE + a * WP: q * PLANE + b * WP],
                    in_=xhl_d.ap()[:, q * PLANE + a * WP:
                                   q * PLANE + b * WP])

            xslice(0, 0)
            xslice(0, 1)
            xslice(1, 0)
            for k in range(2, len(IN_SLICES)):
                xslice(0, k)
                xslice(1, k - 1)
            xslice(1, len(IN_SLICES) - 1)

            # PE p-state warm-up: bf16 matmuls on the identity, all into one
            # PSUM tile (same-engine WAW, no semaphore gaps), no DMA deps.
            make_identity(nc, ident)
            wps = psW.tile([P, 64], F32)
            for _ in range(NWARM):
                nc.tensor.matmul(wps, ident, ident[:, 0:64],
                                 start=True, stop=True)

            # weight APs reused by every chunk
            w_dw = [wpk[:, WDW_O + 256 * i: WDW_O + 256 * (i + 1)]
                    .rearrange("p (k m) -> p k m", k=2) for i in range(5)]
            w_cv = wpk[:, CWB_O: CWB_O + 2 * P].rearrange(
                "p (k m) -> p k m", k=2)
            w_at = _subap(wpk[:, :], ATTW_O, [(0, 2), (1, P)])
            w_ar = wpk[:, ATTW_O + P: ATTW_O + 3 * P].rearrange(
                "p (k m) -> p k m", k=2)

            for band in range(NBANDS):
                a8 = a8p.tile([P, CB_ROWS * W], F8, tag="a8")
                for j in range(NCH):
                    rr = band * CB_ROWS + j * CH_ROWS
                    # ---- depthwise: 5 DoubleRow tap-pair matmuls ----
                    psa = psA.tile([P, 512], F32, tag="A")
                    psa3 = psa.rearrange("p (r c) -> p r c", c=128)
                    for i, (ta, tb) in enumerate(TAP_PAIRS):
                        base = _tap_base(rr, ta)
                        s = _tap_base(rr, tb) - base
                        mv = _subap(xap, base,
                                    [(s, 2), (WP, CH_ROWS), (1, W)])
                        nc.tensor.matmul(psa3, w_dw[i], mv,
                                         start=(i == 0), stop=(i == 4),
                                         perf_mode=PM)
                    # ---- leaky -> fp8 (ACT) ----
                    nc.scalar.activation(a8[:, j * 512: (j + 1) * 512], psa,
                                         AF.Prelu, alpha=NEG)

                    # ---- conv1x1 + attention + residual ----
                    psb = psB.tile([P, 512], F32, tag="B")
                    psb3 = psb.rearrange("p (r c) -> p r c", c=128)
                    nc.tensor.matmul(psb, w_cv,
                                     _subap(a8[:, :], j * 512,
                                            [(0, 2), (1, 512)]),
                                     start=True, stop=False, perf_mode=PM)
                    abase = (rr + 1) * WP + 1
                    nc.tensor.matmul(psb3, w_at,
                                     _subap(xap, abase,
                                            [(PLANE, 2), (WP, CH_ROWS),
                                             (1, W)]),
                                     start=False, stop=False, perf_mode=PM)
                    nc.tensor.matmul(psb3, w_ar,
                                     _subap(xap, abase,
                                            [(0, 2), (WP, CH_ROWS), (1, W)]),
                                     start=False, stop=True, perf_mode=PM)
                    # ---- psum -> bf16 out (+convB) on DVE ----
                    nc.vector.tensor_scalar(outb[:, rr * W: rr * W + 512],
                                            psb, cbf[:, 0:1], None, ALU.add)
                    # half-band output DMA on the (otherwise idle) Pool
                    # queue: a waiting DMA blocks its queue's sequencer, so
                    # it must not share a queue with compute dispatch.
                    if j % 2 == 1:
                        o0 = (rr - CH_ROWS) * W
                        eng = nc.gpsimd if (band * NCH + j) % 4 == 1 \
                            else nc.sync
                        eng.dma_start(
                            out=out_d.ap()[:, o0: o0 + 1024],
                            in_=outb[:, o0: o0 + 1024])

    nc.compile()
    return nc


def _leaky_np(v):
    return np.where(v >= 0, v, NEG * v)


def kernel(x, d, kW1, kW2, convW, convB, caW1, caW2, _trace=False):
    x = np.asarray(x, np.float32)
    d = np.asarray(d, np.float32)
    kW1 = np.asarray(kW1, np.float32)
    kW2 = np.asarray(kW2, np.float32)
    convW = np.asarray(convW, np.float32)
    convB = np.asarray(convB, np.float32)
    caW1 = np.asarray(caW1, np.float32)
    caW2 = np.asarray(caW2, np.float32)
    if "nc" not in _CACHE:
        _CACHE["nc"] = _build()
    nc = _CACHE["nc"]

    # tiny per-sample MLPs on host: kern [B, C, 9], att [B, Cout]
    kern = (_leaky_np(d @ kW1.T) @ kW2.T).reshape(B, C, 9)
    att = 1.0 / (1.0 + np.exp(-(_leaky_np(d @ caW1.T) @ caW2.T)))

    cw8 = convW.astype(NF8).astype(np.float32)
    cwres8 = (convW - cw8).astype(NF8)
    cwb8 = np.zeros((P, 2 * P), NF8)
    for bi in range(BL):
        sl = slice(bi * C, (bi + 1) * C)
        cwb8[sl, bi * C:(bi + 1) * C] = cw8.T.astype(NF8)
        cwb8[sl, P + bi * C: P + (bi + 1) * C] = cwres8.T
    cbf = np.tile(convB, BL)[:, None].astype(np.float32)

    xh = x.astype(NF8)
    xl = (x - xh.astype(np.float32)).astype(NF8)
    k8 = kern.astype(NF8)
    at8 = att.astype(NF8)
    atres8 = (att - at8.astype(np.float32)).astype(NF8)

    in_maps = []
    rng = np.arange(P)
    for c in range(NCORES):
        sl = slice(c * BL, (c + 1) * BL)
        xhl = np.zeros((P, 2, HPAD, WP), NF8)
        xhl[:, 0, 1:H + 1, 1:W + 1] = xh[sl].reshape(P, H, W)
        xhl[:, 1, 1:H + 1, 1:W + 1] = xl[sl].reshape(P, H, W)

        wpk = np.zeros((P, WPK_COLS), NF8)
        kc = k8[sl].reshape(P, 9)
        for t in range(9):
            wpk[rng, WDW_O + t * P + rng] = kc[:, t]
        wpk[rng, ATTW_O + rng] = at8[sl].reshape(P)
        wpk[rng, ATTW_O + P + rng] = atres8[sl].reshape(P)
        wpk[:, CWB_O: CWB_O + 2 * P] = cwb8

        in_maps.append({
            "xhl": np.ascontiguousarray(xhl.reshape(P, 2 * PLANE)),
            "wpk8": wpk,
            "cbf": cbf,
        })

    last_err = None
    for _attempt in range(3):
        try:
            res = run_bass_kernel_spmd(nc, in_maps,
                                       core_ids=list(range(NCORES)),
                                       trace=_trace)
            break
        except Exception as e:  # transient NRT device errors recover on retry
            last_err = e
    else:
        raise last_err
    out = np.concatenate(
        [r["out"].astype(np.float32).reshape(BL, C, H, W)
         for r in res.results], axis=0)
    if _trace:
        return out, res
    return out


# revision 9
# speedup vs baseline: 2.0788x; 1.0344x over previous
"""Trainium2 Bass kernel for nn_DA_conv (dense_cnn).

Model (per batch element b, channel c):
  kern = leaky(d @ kW1.T) @ kW2.T            -> per-(b,c) 3x3 depthwise filter
  dw   = depthwise_conv3x3(x, kern), pad=1   (cross-correlation)
  act  = leaky(dw)
  out  = conv1x1(act, convW) + convB + x * sigmoid-attention(d)

Sharding: data-parallel over batch B=16 across 8 cores (2 images/core).
Per-core layout: 128 SBUF partitions = (2 images x 64 channels).

Strategy (fp8 DoubleRow everywhere on the PE):
- x is split on the host into x_hi = fp8(x) and x_lo = fp8(x - x_hi), both
  uploaded as zero-padded 130x130 planes (fully contiguous DMA, no device
  memsets). The tiny d-MLPs (kern, att) are computed on the host; their
  outputs become fp8 diagonal weight matrices uploaded directly.
- Depthwise: 9 taps = 5 DoubleRow matmuls per 512-px chunk (2 taps per
  matmul via the k-tile dim; moving AP k-tile stride = tap-offset delta).
  x_hi only; kern quantized to fp8 (validated: final rel err ~1.3e-2).
- conv1x1 + attention + residual in 3 DoubleRow matmuls per chunk:
    (convW_hi . a8, convW_res . a8)         stride-0 k-tile on a8
    (att8 . x_hi,   att8 . x_lo)            k-tile stride = plane
    (attres8 . x_hi, 0)                     fp8 residual of sigmoid att
  where a8 = fp8(leaky(dw)) and convW_hi/res is an fp8 hi/lo split.
- ACT does the prelu (psum->fp8), DVE the evacuation (psum->bf16 + bias);
  Pool/gpsimd cannot read PSUM here.
- DMA issue: x planes on SP, weights on ACT, outputs on the gpsimd/Pool
  SWDGE queue (a waiting DMA holds its queue's sequencer, so outputs must
  not share a queue with compute dispatch); the first x slice is small so compute starts
  early; warm-up matmuls on an identity (no DMA dependency) ramp the PE
  p-state while the first DMAs land.
- Output is written bf16 (halves the out-DMA; adds ~4e-4 rel rounding).
"""
import numpy as np
import ml_dtypes

import concourse.bacc as bacc
import concourse.bass as bass
import concourse.mybir as mybir
import concourse.tile as tile
from concourse.bass_utils import run_bass_kernel_spmd
from concourse.masks import make_identity

F32 = mybir.dt.float32
BF16 = mybir.dt.bfloat16
F8 = mybir.dt.float8e4
NF8 = ml_dtypes.float8_e4m3
AF = mybir.ActivationFunctionType
ALU = mybir.AluOpType
PM = mybir.MatmulPerfMode.DoubleRow

B, C, H, W = 16, 64, 128, 128
NCORES = 8
BL = B // NCORES          # images per core (2)
P = BL * C                # partitions used (128)
WP = W + 2                # padded row length (130)
HPAD = H + 2              # padded rows (130)
PLANE = HPAD * WP         # 16900
NEG = 0.1                 # leaky slope

CB_ROWS = 16              # compute band rows
CH_ROWS = 4               # chunk rows (512 psum elements)
NBANDS = H // CB_ROWS     # 8
NCH = CB_ROWS // CH_ROWS  # 4 chunks per band
# padded-row DMA slices (disjoint; first one small so compute starts early)
IN_SLICES = [(0, 12), (12, 34), (34, 56), (56, 78), (78, 100),
             (100, 122), (122, 130)]
NWARM = 50                # PE p-state warm-up matmuls (64 cols each)

# packed fp8 weights blob layout: wdw | attw | cwb8
WDW_O, ATTW_O, CWB_O = 0, 10 * P, 13 * P
WPK_COLS = 15 * P

# depthwise tap pairs: (first tap, second tap); t = 3*(dy+1) + (dx+1)
TAP_PAIRS = [(0, 1), (2, 3), (4, 5), (6, 7), (8, 8)]

_CACHE = {}


def _tap_base(rr, t):
    dy, dx = t // 3 - 1, t % 3 - 1
    return (rr + 1 + dy) * WP + 1 + dx


def _subap(apx, off, dims):
    part = list(list(apx.ap)[0])
    return bass.AP(apx.tensor, apx.offset + off,
                   [part] + [list(d) for d in dims])


def _build():
    nc = bacc.Bacc("TRN2", target_bir_lowering=False, debug=False)

    xhl_d = nc.dram_tensor("xhl", [P, 2 * PLANE], F8, kind="ExternalInput")
    wpk_d = nc.dram_tensor("wpk8", [P, WPK_COLS], F8, kind="ExternalInput")
    cbf_d = nc.dram_tensor("cbf", [P, 1], F32, kind="ExternalInput")
    out_d = nc.dram_tensor("out", [P, H * W], BF16, kind="ExternalOutput")

    with tile.TileContext(nc) as tc:
        with (
            tc.tile_pool(name="consts", bufs=1) as consts,
            tc.tile_pool(name="a8p", bufs=4) as a8p,
            tc.tile_pool(name="psA", bufs=4, space="PSUM") as psA,
            tc.tile_pool(name="psB", bufs=4, space="PSUM") as psB,
        ):
            xhl = consts.tile([P, 2 * PLANE], F8)
            xap = xhl[:, :]
            wpk = consts.tile([P, WPK_COLS], F8)
            cbf = consts.tile([P, 1], F32)
            ident = consts.tile([P, P], BF16)
            outb = consts.tile([P, H * W], BF16)

            # weights first (they gate the first matmul), then x slices.
            # hi slices run one ahead of lo: the depthwise only needs hi,
            # the conv/att pass needs lo slightly later.
            nc.sync.dma_start(out=wpk, in_=wpk_d.ap())

            def xslice(q, k):
                (a, b) = IN_SLICES[k]
                nc.sync.dma_start(
                    out=xhl[:, q * PLANE + a * WP: q * PLANE + b * WP],
                    in_=xhl_d.ap()[:, q * PLANE + a * WP:
                                   q * PLANE + b * WP])

            xslice(0, 0)
            xslice(0, 1)
            xslice(1, 0)
            xslice(1, 1)
            nc.sync.dma_start(out=cbf, in_=cbf_d.ap())
            for k in range(2, len(IN_SLICES)):
                xslice(0, k)
                xslice(1, k - 1)
            xslice(1, len(IN_SLICES) - 1)

            # PE p-state warm-up: bf16 matmuls on the identity, all into one
            # PSUM tile (same-engine WAW, no semaphore gaps), no DMA deps.
            make_identity(nc, ident)
            wps = psB.tile([P, 64], F32, tag="B")
            for _ in range(NWARM):
                nc.tensor.matmul(wps, ident, ident[:, 0:64],
                                 start=True, stop=True)

            # weight APs reused by every chunk
            w_dw = [wpk[:, WDW_O + 256 * i: WDW_O + 256 * (i + 1)]
                    .rearrange("p (k m) -> p k m", k=2) for i in range(5)]
            w_cv = wpk[:, CWB_O: CWB_O + 2 * P].rearrange(
                "p (k m) -> p k m", k=2)
            w_at = _subap(wpk[:, :], ATTW_O, [(0, 2), (1, P)])
            w_ar = wpk[:, ATTW_O + P: ATTW_O + 3 * P].rearrange(
                "p (k m) -> p k m", k=2)

            for band in range(NBANDS):
                a8 = a8p.tile([P, CB_ROWS * W], F8, tag="a8")
                for j in range(NCH):
                    rr = band * CB_ROWS + j * CH_ROWS
                    # ---- depthwise: 5 DoubleRow tap-pair matmuls ----
                    psa = psA.tile([P, 512], F32, tag="A")
                    psa3 = psa.rearrange("p (r c) -> p r c", c=128)
                    for i, (ta, tb) in enumerate(TAP_PAIRS):
                        base = _tap_base(rr, ta)
                        s = _tap_base(rr, tb) - base
                        mv = _subap(xap, base,
                                    [(s, 2), (WP, CH_ROWS), (1, W)])
                        nc.tensor.matmul(psa3, w_dw[i], mv,
                                         start=(i == 0), stop=(i == 4),
                                         perf_mode=PM)
                    # ---- leaky -> fp8 (ACT) ----
                    nc.scalar.activation(a8[:, j * 512: (j + 1) * 512], psa,
                                         AF.Prelu, alpha=NEG)

                    # ---- conv1x1 + attention + residual ----
                    psb = psB.tile([P, 512], F32, tag="B")
                    psb3 = psb.rearrange("p (r c) -> p r c", c=128)
                    nc.tensor.matmul(psb, w_cv,
                                     _subap(a8[:, :], j * 512,
                                            [(0, 2), (1, 512)]),
                                     start=True, stop=False, perf_mode=PM)
                    abase = (rr + 1) * WP + 1
                    nc.tensor.matmul(psb3, w_at,
                                     _subap(xap, abase,
                                            [(PLANE, 2), (WP, CH_ROWS),
                                             (1, W)]),
                                     start=False, stop=False, perf_mode=PM)
                    nc.tensor.matmul(psb3, w_ar,
                                     _subap(xap, abase,
                                            [(0, 2), (WP, CH_ROWS), (1, W)]),
                                     start=False, stop=True, perf_mode=PM)
                    # ---- psum -> bf16 out (+convB) on DVE ----
                    nc.vector.tensor_scalar(outb[:, rr * W: rr * W + 512],
                                            psb, cbf[:, 0:1], None, ALU.add)
                    # output DMAs ride the Pool (SWDGE) and SP queues —
                    # a waiting DMA blocks its queue's sequencer, so they
                    # must not share a queue with compute dispatch. The
                    # last band issues per-chunk DMAs to shorten the tail.
                    if band == NBANDS - 1:
                        o0 = rr * W
                        eng = nc.gpsimd if j % 2 == 1 else nc.sync
                        eng.dma_start(out=out_d.ap()[:, o0: o0 + 512],
                                      in_=outb[:, o0: o0 + 512])
                    elif j % 2 == 1:
                        o0 = (rr - CH_ROWS) * W
                        eng = nc.gpsimd if (band * NCH + j) % 4 == 1 \
                            else nc.sync
                        eng.dma_start(
                            out=out_d.ap()[:, o0: o0 + 1024],
                            in_=outb[:, o0: o0 + 1024])

    nc.compile()
    return nc


def _leaky_np(v):
    return np.where(v >= 0, v, NEG * v)


def kernel(x, d, kW1, kW2, convW, convB, caW1, caW2, _trace=False):
    x = np.asarray(x, np.float32)
    d = np.asarray(d, np.float32)
    kW1 = np.asarray(kW1, np.float32)
    kW2 = np.asarray(kW2, np.float32)
    convW = np.asarray(convW, np.float32)
    convB = np.asarray(convB, np.float32)
    caW1 = np.asarray(caW1, np.float32)
    caW2 = np.asarray(caW2, np.float32)
    if "nc" not in _CACHE:
        _CACHE["nc"] = _build()
    nc = _CACHE["nc"]

    # tiny per-sample MLPs on host: kern [B, C, 9], att [B, Cout]
    kern = (_leaky_np(d @ kW1.T) @ kW2.T).reshape(B, C, 9)
    att = 1.0 / (1.0 + np.exp(-(_leaky_np(d @ caW1.T) @ caW2.T)))

    cw8 = convW.astype(NF8).astype(np.float32)
    cwres8 = (convW - cw8).astype(NF8)
    cwb8 = np.zeros((P, 2 * P), NF8)
    for bi in range(BL):
        sl = slice(bi * C, (bi + 1) * C)
        cwb8[sl, bi * C:(bi + 1) * C] = cw8.T.astype(NF8)
        cwb8[sl, P + bi * C: P + (bi + 1) * C] = cwres8.T
    cbf = np.tile(convB, BL)[:, None].astype(np.float32)

    xh = x.astype(NF8)
    xl = (x - xh.astype(np.float32)).astype(NF8)
    k8 = kern.astype(NF8)
    at8 = att.astype(NF8)
    atres8 = (att - at8.astype(np.float32)).astype(NF8)

    in_maps = []
    rng = np.arange(P)
    for c in range(NCORES):
        sl = slice(c * BL, (c + 1) * BL)
        xhl = np.zeros((P, 2, HPAD, WP), NF8)
        xhl[:, 0, 1:H + 1, 1:W + 1] = xh[sl].reshape(P, H, W)
        xhl[:, 1, 1:H + 1, 1:W + 1] = xl[sl].reshape(P, H, W)

        wpk = np.zeros((P, WPK_COLS), NF8)
        kc = k8[sl].reshape(P, 9)
        for t in range(9):
            wpk[rng, WDW_O + t * P + rng] = kc[:, t]
        wpk[rng, ATTW_O + rng] = at8[sl].reshape(P)
        wpk[rng, ATTW_O + P + rng] = atres8[sl].reshape(P)
        wpk[:, CWB_O: CWB_O + 2 * P] = cwb8

        in_maps.append({
            "xhl": np.ascontiguousarray(xhl.reshape(P, 2 * PLANE)),
            "wpk8": wpk,
            "cbf": cbf,
        })

    last_err = None
    for _attempt in range(3):
        try:
            res = run_bass_kernel_spmd(nc, in_maps,
                                       core_ids=list(range(NCORES)),
                                       trace=_trace)
            break
        except Exception as e:  # transient NRT device errors recover on retry
            last_err = e
    else:
        raise last_err
    out = np.concatenate(
        [r["out"].astype(np.float32).reshape(BL, C, H, W)
         for r in res.results], axis=0)
    if _trace:
        return out, res
    return out
